# revision 1
# baseline (speedup 1.0000x reference)
"""Trainium2 Bass kernel for nn_AttentionHead (B=4, S=4096, H=1024, D=64).

Reference computation (note the unusual K-first ordering):
    K = x @ Wk.T; Q = x @ Wq.T; V = x @ Wv.T            [B,S,D]
    scores[b,i,j] = (K[b,i] . Q[b,j]) / sqrt(D)         [B,S,S]
    scores[:, :, j] = -1e12 where mask[:, j] == 0
    out = softmax(scores, axis=2) @ V                   [B,S,S] @ [B,S,D]

Key structural choices:
  - Masked j-columns get softmax weight EXACTLY 0 (exp underflows), so the
    host drops them up front: the query/value axis is compacted from the
    mask (~2048 of 4096 survive) and padded to a fixed J (2304 by default;
    the build is parameterized on J as a fallback for denser masks). This
    halves the scores/exp/AV work, which dominates.
  - x^T in bf16 is pure data movement, so the host ships it pre-transposed
    (like the baseline's host-side roll): no on-chip transposes of x, no
    fp32->bf16 casts, and half the HBM traffic. Weights/identities are
    host-cast too, so no DMA needs the (slow, gpsimd-only) cast path and
    bulk input streams across all three DMA trigger queues (gpsimd SWDGE +
    sync/scalar HWDGE).
  - Scores use PE row tiling: contraction is only D=64, so the two 512-wide
    score matmuls of a slot run CONCURRENTLY on row groups 0/1 of the PE
    array (~2x on the scores leg, and their LDWEIGHTS overlap in-flight
    matmuls of the other group). This requires Q^T and K^T duplicated into
    partitions 64:128: K^T comes for free from a [Wk|Wk] stationary
    projection; Q^T via one DVE partition-shift copy per block.

Sharding: 8 cores = 4 batches x 2 key-row halves of 2048. Each core gets
x^T for its own 2048 key rows (xtk) plus the batch-shared mask-compacted
x^T for queries/values (xtq).

Per-core pipeline (bf16 matmuls, fp32 accumulation):
  - One [Wq|Wv] stationary gives Q^T (rows 0:64) and V^T (rows 64:128) per
    query-column block; [Wk|Wk] gives duplicated K^T over own 2048 rows;
    V^T -> V via PE transposes. V gets a ones column (softmax denominator).
  - PE warmup matmuls on junk data cover the DMA ramp so the HAM
    clock-gate sits at 8/8 when real work arrives.
  - Two passes over query tiles t=0..JT-1 (one per 1024-wide i-half). Per
    slot: scores^T = Q^T_t.T @ K^T on PE (row-tiled pair); exp(0.125*s +
    maskbias[j]) on ACT (mask/pad folded into the per-partition bias;
    masked queries underflow to exactly 0); PE accumulates V'_t.T @ P^T_t
    into out'^T [65, 1024] - rows 0:64 numerator^T, row 64 denominator.
    The AV matmuls are emitted one slot BEHIND the scores matmuls:
    otherwise they head-of-line block the PE queue waiting on exp. Pass A
    is emission-interleaved with the projection stream so PE always has
    dense work chasing the DMA; pass-B-only data (xtk cols 1024:2048) is
    deprioritized in the DMA queues.
  - Per-pass finale: copy acc to SBUF (freeing the PSUM accumulator for
    pass B), then 128-col pieces: PE-transpose via identity matmul, out =
    numerator * reciprocal(denominator) on DVE, store every 256 rows as
    soon as ready. Pass A's finale pieces are interleaved into pass B's
    early slots where the PE has slack (pass B is exp/ACT-paced), and the
    pass-A-to-B boundary never serializes: the deferred AV entry carries
    its own accumulator and flushes inside pass B's first slot.
"""

import numpy as np

B, S, H, D = 4, 4096, 1024, 64
N_CORES = 8
SC = S // 2  # key rows (output rows) per core
HC = H // 128  # contraction chunks
J_MIN = 1024  # floor for the padded, mask-compacted query-column count
NEG = -30000.0
N_WARM = 40

_CACHE = {}


def _build(J, nfull):
    # nfull: query tiles [0, nfull) are fully kept for EVERY batch, so their
    # exp needs no mask bias (saves the ACT per-partition bias read).
    import concourse.tile as tile
    from concourse import bacc, mybir

    dt = mybir.dt
    AF = mybir.ActivationFunctionType
    JT = J // 128
    qblocks = [(c0, min(c0 + 512, J)) for c0 in range(0, J, 512)]

    nc = bacc.Bacc(
        "TRN2", target_bir_lowering=False, debug=False, num_devices=N_CORES
    )
    xtk = nc.dram_tensor("xtk", [H, SC], dt.bfloat16, kind="ExternalInput").ap()
    xtq = nc.dram_tensor("xtq", [H, J], dt.bfloat16, kind="ExternalInput").ap()
    wqv = nc.dram_tensor("wqv", [H, 2 * D], dt.bfloat16, kind="ExternalInput").ap()
    wkk = nc.dram_tensor("wkk", [H, 2 * D], dt.bfloat16, kind="ExternalInput").ap()
    mb = nc.dram_tensor("mb", [128, JT], dt.float32, kind="ExternalInput").ap()
    idb = nc.dram_tensor("idb", [128, 128], dt.bfloat16, kind="ExternalInput").ap()
    idf = nc.dram_tensor("idf", [D + 1, D + 1], dt.float32, kind="ExternalInput").ap()
    out = nc.dram_tensor("out", [SC, D], dt.float32, kind="ExternalOutput").ap()

    xtk_r = xtk.rearrange("(c p) s -> p c s", p=128)
    xtq_r = xtq.rearrange("(c p) s -> p c s", p=128)

    with (
        tile.TileContext(nc) as tc,
        tc.tile_pool(name="persist", bufs=1) as persist,
        tc.tile_pool(name="ptile", bufs=6) as ptile,
        tc.tile_pool(name="accs", bufs=2) as accs,
        tc.tile_pool(name="fin", bufs=2) as fin,
    ):
        qt = persist.tile([128, J], dt.bfloat16)  # Q^T duplicated rows 0:64/64:128
        kt = persist.tile([128, SC], dt.bfloat16)  # K^T duplicated rows 0:64/64:128
        vtsb = persist.tile([128, J], dt.bfloat16)  # rows 64:128 = V^T
        vp = persist.tile([128, JT, D + 1], dt.bfloat16)
        mb_sb = persist.tile([128, JT], dt.float32)
        idf_sb = persist.tile([D + 1, D + 1], dt.float32)
        idb_sb = persist.tile([128, 128], dt.bfloat16)
        wtile = persist.tile([128, 512], dt.bfloat16)
        xk_sb = persist.tile([128, HC, SC], dt.bfloat16)
        xq_sb = persist.tile([128, HC, J], dt.bfloat16)
        wqv_sb = persist.tile([128, HC, 2 * D], dt.bfloat16)
        wkk_sb = persist.tile([128, HC, 2 * D], dt.bfloat16)

        nc.vector.memset(vp[:, :, D], 1.0)
        nc.vector.memset(wtile[:], 0.0)

        with (
            tc.tile_pool(name="psco", bufs=2, space="PSUM") as psco,
            tc.tile_pool(name="ppx", bufs=2, space="PSUM") as ppx,
            tc.tile_pool(name="pacc", bufs=1, space="PSUM") as pacc,
        ):
            # --- DMA queue plans; pass-A-critical data first on each queue ---
            def big_loads():
                # All three trigger engines pump concurrently (~1/3 of HBM
                # bandwidth each); units are [128, 4, 512] H-chunk halves
                # (1KB contiguous lines) round-robined in global priority
                # order: kt's prerequisites first, pass-B-only xtk cols
                # 1024:2048 last. Measured best overall even though exp(0)
                # waits out the scalar queue's share of the transfers.
                nc.gpsimd.dma_start(
                    wkk_sb[:], wkk.rearrange("(c p) d -> p c d", p=128)
                )
                nc.scalar.dma_start(
                    wqv_sb[:], wqv.rearrange("(c p) d -> p c d", p=128)
                )
                nc.sync.dma_start(mb_sb[:], mb[:])
                nc.sync.dma_start(idf_sb[:], idf[:])
                nc.sync.dma_start(idb_sb[:], idb[:])
                qs = [nc.sync, nc.gpsimd, nc.scalar]
                qi = 0

                def unit(dst, src, c0, c1):
                    nonlocal qi
                    for h0, h1 in ((0, 4), (4, 8)):
                        qs[qi % 3].dma_start(
                            dst[:, h0:h1, c0:c1], src[:, h0:h1, c0:c1]
                        )
                        qi += 1

                for c in range(0, 1024, 512):
                    unit(xk_sb, xtk_r, c, c + 512)
                for c in range(0, J, 512):
                    unit(xq_sb, xtq_r, c, min(c + 512, J))
                for c in range(1024, 2048, 512):
                    unit(xk_sb, xtk_r, c, c + 512)

            # --- PE work generators ---
            def proj_qv(bi):  # [Q^T; V^T] for one query-column block
                c0, c1 = qblocks[bi]
                ps = ppx.tile([128, c1 - c0], dt.float32, tag="px")
                for hc in range(HC):
                    nc.tensor.matmul(
                        ps[:],
                        wqv_sb[:, hc, :],
                        xq_sb[:, hc, c0:c1],
                        start=(hc == 0),
                        stop=(hc == HC - 1),
                    )
                nc.vector.tensor_copy(qt[0:64, c0:c1], ps[0:64, :])
                nc.vector.tensor_copy(vtsb[64:128, c0:c1], ps[64:128, :])
                # duplicate Q^T into partitions 64:128 for row-tiled scores
                nc.vector.tensor_copy(qt[64:128, c0:c1], ps[0:64, :])

            def proj_k(sb):  # [Wk|Wk] stationary -> K^T in both halves
                ps = ppx.tile([128, 512], dt.float32, tag="px")
                for hc in range(HC):
                    nc.tensor.matmul(
                        ps[:],
                        wkk_sb[:, hc, :],
                        xk_sb[:, hc, 512 * sb : 512 * (sb + 1)],
                        start=(hc == 0),
                        stop=(hc == HC - 1),
                    )
                nc.vector.tensor_copy(kt[:, 512 * sb : 512 * (sb + 1)], ps[:])

            def vt_block(st0, st1):  # V^T -> V via PE transpose
                for st in range(st0, st1):
                    pvt = ppx.tile([128, D], dt.bfloat16, tag="px")
                    nc.tensor.transpose(
                        pvt[:],
                        vtsb[64:128, 128 * st : 128 * (st + 1)],
                        idb_sb[64:128, 64:128],
                    )
                    nc.vector.tensor_copy(vp[:, st, 0:D], pvt[:])

            # --- t-loop slot machinery: AV deferred one slot behind ---
            pending = []

            def flush_av():
                if not pending:
                    return
                pt, t, acc = pending.pop()
                for nb in range(2):
                    nc.tensor.matmul(
                        acc[:, 512 * nb : 512 * (nb + 1)],
                        vp[:, t, :],
                        pt[:, 512 * nb : 512 * (nb + 1)],
                        start=(t == 0),
                        stop=(t == JT - 1),
                    )

            def t_slot(t, acc, ih):
                ps = psco.tile([128, 1024], dt.float32, tag="ps")
                # row-tiled pair: groups 0/1 run concurrently (contraction 64)
                nc.tensor.matmul(
                    ps[:, 0:512],
                    qt[0:64, 128 * t : 128 * (t + 1)],
                    kt[0:64, 1024 * ih : 1024 * ih + 512],
                    start=True,
                    stop=True,
                )
                nc.tensor.matmul(
                    ps[:, 512:1024],
                    qt[64:128, 128 * t : 128 * (t + 1)],
                    kt[64:128, 1024 * ih + 512 : 1024 * ih + 1024],
                    start=True,
                    stop=True,
                )
                flush_av()
                pt = ptile.tile([128, 1024], dt.bfloat16)
                if t < nfull:
                    nc.scalar.activation(pt[:], ps[:], AF.Exp, scale=0.125)
                else:
                    nc.scalar.activation(
                        pt[:], ps[:], AF.Exp, bias=mb_sb[:, t : t + 1], scale=0.125
                    )
                pending.append((pt, t, acc))

            def acc_to_sb(acc):  # copy PSUM acc to SBUF, freeing pacc
                acc_sb = accs.tile([D + 1, 1024], dt.float32, tag="accs")
                nc.vector.tensor_copy(acc_sb[:, 0:512], acc[:, 0:512])
                nc.vector.tensor_copy(acc_sb[:, 512:1024], acc[:, 512:1024])
                return acc_sb

            def finale_piece(acc_sb, ih, k):  # one 128-col chunk
                po = ppx.tile([128, D + 1], dt.float32, tag="px")
                nc.tensor.transpose(
                    po[:], acc_sb[:, 128 * k : 128 * (k + 1)], idf_sb[:]
                )
                rc = fin.tile([128, 1], dt.float32, tag="rc")
                nc.vector.reciprocal(rc[:], po[:, D : D + 1])
                nc.vector.tensor_scalar_mul(
                    oall[:, 8 * ih + k, :], po[:, 0:D], rc[:]
                )
                if k % 2 == 1:  # store every 256 rows as soon as ready
                    r0 = 1024 * ih + 128 * (k - 1)
                    nc.sync.dma_start(
                        out[r0 : r0 + 256, :].rearrange("(k p) d -> p k d", p=128),
                        oall[:, 8 * ih + k - 1 : 8 * ih + k + 1, :],
                    )

            oall = fin.tile([128, 16, D], dt.float32, tag="oall")

            # ---- pass A (i-half 0) interleaved with the projections ----
            big_loads()
            accA = pacc.tile([D + 1, 1024], dt.float32, tag="acc")
            tA = lambda t: t_slot(t, accA, 0)
            # PE warmup while the first slices stream in
            pw = ppx.tile([128, 512], dt.float32, tag="px")
            for _ in range(N_WARM):
                nc.tensor.matmul(
                    pw[:], wtile[:, 0:128], wtile[:], start=True, stop=True
                )
            dummy = fin.tile([128, 1], dt.float32, tag="dummy")
            nc.scalar.activation(dummy[:], wtile[:, 0:1], AF.Exp)
            proj_k(0)
            proj_k(1)
            proj_qv(0)
            vt_cover = qblocks[0][1] // 128
            vt_block(0, vt_cover)
            next_t = 0
            units = [("qv", i) for i in range(1, len(qblocks))]
            units += [("k", 2), ("k", 3)]
            for kind, i in units:
                # emit already-runnable slots BEFORE the next proj unit:
                # the PE queue is in-order, so a proj waiting on its DMA
                # must not head-of-line-block ready slots
                tgt = min(vt_cover, next_t + 2)
                while next_t < tgt:
                    tA(next_t)
                    next_t += 1
                if kind == "qv":
                    proj_qv(i)
                    new_cover = qblocks[i][1] // 128
                    vt_block(vt_cover, new_cover)
                    vt_cover = new_cover
                else:
                    proj_k(i)
            while next_t < JT:
                tA(next_t)
                next_t += 1
            # A's last AV stays pending: it flushes inside pass B's slot 0,
            # so pass-B scores aren't serialized behind it

            # ---- pass B (i-half 1), finale A interleaved into its slack ----
            accB = pacc.tile([D + 1, 1024], dt.float32, tag="acc")
            fa = 0
            acc_sbA = None
            for t in range(JT):
                t_slot(t, accB, 1)
                if t == 0:
                    acc_sbA = acc_to_sb(accA)  # frees pacc banks for accB
                elif fa < 8:
                    finale_piece(acc_sbA, 0, fa)
                    fa += 1
            while fa < 8:
                finale_piece(acc_sbA, 0, fa)
                fa += 1
            flush_av()
            acc_sbB = acc_to_sb(accB)
            for k in range(8):
                finale_piece(acc_sbB, 1, k)

    nc.compile()
    return nc


def _in_maps(x, mask, Wk, Wq, Wv):
    import ml_dtypes

    bf16 = ml_dtypes.bfloat16
    wqv = np.ascontiguousarray(
        np.concatenate([Wq.T, Wv.T], axis=1).astype(bf16)
    )
    wkk = np.ascontiguousarray(np.concatenate([Wk.T, Wk.T], axis=1).astype(bf16))
    idb = np.eye(128, dtype=bf16)
    idf = np.eye(D + 1, dtype=np.float32)
    nk = [int((mask[b] != 0).sum()) for b in range(B)]
    J = max(J_MIN, -(-max(nk) // 128) * 128)
    nfull = min(nk) // 128
    JT = J // 128
    xtq_b, mb_b = [], []
    for b in range(B):
        idx = np.flatnonzero(mask[b] != 0)
        xt = np.zeros((H, J), dtype=bf16)
        xt[:, : len(idx)] = x[b].T[:, idx].astype(bf16)
        xtq_b.append(xt)
        mbv = np.full(J, np.float32(NEG), dtype=np.float32)
        mbv[: len(idx)] = 0.0
        mb_b.append(np.ascontiguousarray(mbv.reshape(JT, 128).T))
    maps = []
    for c in range(N_CORES):
        b, half = c // 2, c % 2
        xtk = np.ascontiguousarray(x[b, half * SC : (half + 1) * SC].T.astype(bf16))
        maps.append(
            {
                "xtk": xtk,
                "xtq": xtq_b[b],
                "wqv": wqv,
                "wkk": wkk,
                "mb": mb_b[b],
                "idb": idb,
                "idf": idf,
            }
        )
    return maps, (J, nfull)


def kernel(x, mask, Wk, Wq, Wv):
    from concourse.bass_utils import run_bass_kernel_spmd

    maps, key = _in_maps(x, mask, Wk, Wq, Wv)
    if key not in _CACHE:
        _CACHE[key] = _build(*key)
    nc = _CACHE[key]
    br = run_bass_kernel_spmd(nc, maps, list(range(N_CORES)))
    out = np.empty((B, S, D), dtype=np.float32)
    for c in range(N_CORES):
        b, half = c // 2, c % 2
        out[b, half * SC : (half + 1) * SC, :] = br.results[c]["out"]
    return out



# revision 6
# speedup vs baseline: 1.0330x; 1.0330x over previous
"""Trainium2 Bass kernel for nn_AttentionHead (B=4, S=4096, H=1024, D=64).

Reference computation (note the unusual K-first ordering):
    K = x @ Wk.T; Q = x @ Wq.T; V = x @ Wv.T            [B,S,D]
    scores[b,i,j] = (K[b,i] . Q[b,j]) / sqrt(D)         [B,S,S]
    scores[:, :, j] = -1e12 where mask[:, j] == 0
    out = softmax(scores, axis=2) @ V                   [B,S,S] @ [B,S,D]

Key structural choices (v2 — DMA/fill-phase and finale overhaul):
  - Masked j-columns get softmax weight EXACTLY 0 (exp underflows), so the
    host drops them up front: the query/value axis is compacted from the
    mask (~2048 of 4096 survive) and padded to a fixed J. This halves the
    scores/exp/AV work, which dominates.
  - x^T ships pre-transposed in bf16; all weights ship as one merged
    [Wq|Wv|Wk|Wk] tensor (one stationary gives Q^T+V^T, the other K^T
    duplicated for PE row tiling).
  - Scores use PE row tiling (contraction D=64): the two 512-wide score
    matmuls of a slot run concurrently on row groups 0/1.
  - The t-loop is ACT(exp)-paced (~1.3us per [128,1024] tile); everything
    else is scheduled to never stall the scalar engine: its queue carries
    only the dummy exp + two tiny loads.
  - DMA: five trigger queues (sync/scalar/gpsimd/vector/tensor). The
    critical fill set (w4, xk half A, xq block 0) is spread across queues
    so it lands in parallel with >=2KB lines; pass-B-only data (xk cols
    1024:2048) and late xq blocks queue behind. The warmup is 128-wide
    junk matmuls bridging trigger-issue to first-data.
  - AV matmuls are emitted one slot behind scores (PE queue is in-order;
    they must not head-of-line block waiting on exp).
  - Finale v2: out^T stays in transposed space. The softmax denominator
    row is broadcast to 64 partitions with a K=1 matmul against a ones
    stationary, reciprocal+multiply on DVE, and the [64, 1024] fp32 result
    is stored as out^T with 4KB lines (the host transposes back). This
    replaces 16 PE transposes + 2048-descriptor stores.

Sharding: 8 cores = 4 batches x 2 key-row halves of 2048. Core output is
out^T [64, 2048]; host reassembles.
"""

import numpy as np

B, S, H, D = 4, 4096, 1024, 64
N_CORES = 8
SC = S // 2  # key rows (output rows) per core
HC = H // 128  # contraction chunks
J_MIN = 1024  # floor for the padded, mask-compacted query-column count
NEG = -30000.0
N_WARM = 40

_CACHE = {}


def _build(J, nfull):
    # nfull: query tiles [0, nfull) are fully kept for EVERY batch, so their
    # exp needs no mask bias (saves the ACT per-partition bias read).
    import concourse.tile as tile
    from concourse import bacc, mybir

    dt = mybir.dt
    AF = mybir.ActivationFunctionType
    JT = J // 128
    qblocks = [(c0, min(c0 + 512, J)) for c0 in range(0, J, 512)]

    nc = bacc.Bacc(
        "TRN2", target_bir_lowering=False, debug=False, num_devices=N_CORES
    )
    xtk = nc.dram_tensor("xtk", [H, SC], dt.bfloat16, kind="ExternalInput").ap()
    xtq = nc.dram_tensor("xtq", [H, J], dt.bfloat16, kind="ExternalInput").ap()
    w4 = nc.dram_tensor("w4", [H, 4 * D], dt.bfloat16, kind="ExternalInput").ap()
    mb = nc.dram_tensor("mb", [128, JT], dt.float32, kind="ExternalInput").ap()
    idb = nc.dram_tensor("idb", [128, 128], dt.bfloat16, kind="ExternalInput").ap()
    outt = nc.dram_tensor("outt", [D, SC], dt.float32, kind="ExternalOutput").ap()

    xtk_r = xtk.rearrange("(c p) s -> p c s", p=128)
    xtq_r = xtq.rearrange("(c p) s -> p c s", p=128)
    w4_r = w4.rearrange("(c p) d -> p c d", p=128)

    with (
        tile.TileContext(nc) as tc,
        tc.tile_pool(name="persist", bufs=1) as persist,
        tc.tile_pool(name="ptile", bufs=6) as ptile,
    ):
        qt = persist.tile([128, J], dt.bfloat16)  # Q^T duplicated rows 0:64/64:128
        kt = persist.tile([128, SC], dt.bfloat16)  # K^T duplicated rows 0:64/64:128
        vtsb = persist.tile([128, J], dt.bfloat16)  # rows 64:128 = V^T
        vp = persist.tile([128, JT, D + 1], dt.bfloat16)
        mb_sb = persist.tile([128, JT], dt.float32)
        idb_sb = persist.tile([128, 128], dt.bfloat16)
        wsb = persist.tile([128, HC, 4 * D], dt.bfloat16)
        xk_sb = persist.tile([128, HC, SC], dt.bfloat16)
        xq_sb = persist.tile([128, HC, J], dt.bfloat16)
        onesb = persist.tile([1, D], dt.float32)
        wtile = persist.tile([128, 128], dt.bfloat16)
        acc_sb = persist.tile([D + 1, 2, 1024], dt.float32)
        den_sb = persist.tile([1, 2, 1024], dt.float32)
        rc_sb = persist.tile([D, 2, 1024], dt.float32)
        outT = persist.tile([D, 2, 1024], dt.float32)

        nc.vector.memset(wtile[:], 0.0)
        nc.vector.memset(vp[:, :, D], 1.0)
        nc.vector.memset(onesb[:], 1.0)

        with (
            tc.tile_pool(name="psco", bufs=2, space="PSUM") as psco,
            tc.tile_pool(name="ppx", bufs=2, space="PSUM") as ppx,
            tc.tile_pool(name="pacc", bufs=1, space="PSUM") as pacc,
        ):
            dummy = persist.tile([128, 1], dt.float32)
            nc.scalar.activation(dummy[:], wtile[:, 0:1], AF.Exp)

            # --- DMA queue plans ---
            # Critical fill set spread across queues; scalar stays light so
            # the exp stream never waits on trigger instructions.
            xqranges = [(c0, min(c0 + 1024, J)) for c0 in range(0, J, 1024)]

            def big_loads():
                # critical fill set (w4, xk half A, xq range 0) balanced
                # across the three queues, in parallel, 2KB+ lines; late
                # data (xq ranges 1+, xk half B) queues behind on
                # sync/gpsimd so the scalar engine is free for exp early.
                nc.scalar.dma_start(mb_sb[:], mb[:])
                nc.scalar.dma_start(idb_sb[:], idb[:])
                nc.sync.dma_start(wsb[:, 0:4, :], w4_r[:, 0:4, :])
                nc.gpsimd.dma_start(wsb[:, 4:8, :], w4_r[:, 4:8, :])
                nc.scalar.dma_start(xk_sb[:, 0:3, 0:1024], xtk_r[:, 0:3, 0:1024])
                nc.sync.dma_start(xk_sb[:, 3:5, 0:1024], xtk_r[:, 3:5, 0:1024])
                nc.gpsimd.dma_start(xk_sb[:, 5:8, 0:1024], xtk_r[:, 5:8, 0:1024])
                r0, r1 = xqranges[0]
                nc.scalar.dma_start(xq_sb[:, 0:3, r0:r1], xtq_r[:, 0:3, r0:r1])
                nc.sync.dma_start(xq_sb[:, 3:5, r0:r1], xtq_r[:, 3:5, r0:r1])
                nc.gpsimd.dma_start(xq_sb[:, 5:8, r0:r1], xtq_r[:, 5:8, r0:r1])
                lateq = [nc.sync, nc.gpsimd]
                for i, (c0, c1) in enumerate(xqranges[1:]):
                    lateq[i % 2].dma_start(
                        xq_sb[:, 0:4, c0:c1], xtq_r[:, 0:4, c0:c1]
                    )
                    lateq[(i + 1) % 2].dma_start(
                        xq_sb[:, 4:8, c0:c1], xtq_r[:, 4:8, c0:c1]
                    )
                nc.sync.dma_start(xk_sb[:, 0:4, 1024:2048], xtk_r[:, 0:4, 1024:2048])
                nc.gpsimd.dma_start(xk_sb[:, 4:8, 1024:2048], xtk_r[:, 4:8, 1024:2048])

            # --- PE work generators ---
            def proj_k_pair(sb):  # kt blocks [1024*sb, 1024*sb+1024), per-hc
                c0 = 1024 * sb
                psL = ppx.tile([128, 512], dt.float32, tag="px")
                psR = ppx.tile([128, 512], dt.float32, tag="px")
                for hc in range(HC):
                    nc.tensor.matmul(
                        psL[:],
                        wsb[:, hc, 128:256],
                        xk_sb[:, hc, c0 : c0 + 512],
                        start=(hc == 0),
                        stop=(hc == HC - 1),
                    )
                    nc.tensor.matmul(
                        psR[:],
                        wsb[:, hc, 128:256],
                        xk_sb[:, hc, c0 + 512 : c0 + 1024],
                        start=(hc == 0),
                        stop=(hc == HC - 1),
                    )
                nc.vector.tensor_copy(kt[:, c0 : c0 + 512], psL[:])
                nc.vector.tensor_copy(kt[:, c0 + 512 : c0 + 1024], psR[:])

            def proj_qv(bi):  # [Q^T; V^T] for one query-column block
                c0, c1 = qblocks[bi]
                ps = ppx.tile([128, c1 - c0], dt.float32, tag="px")
                for hc in range(HC):
                    nc.tensor.matmul(
                        ps[:],
                        wsb[:, hc, 0:128],
                        xq_sb[:, hc, c0:c1],
                        start=(hc == 0),
                        stop=(hc == HC - 1),
                    )
                nc.vector.tensor_copy(qt[0:64, c0:c1], ps[0:64, :])
                nc.vector.tensor_copy(vtsb[64:128, c0:c1], ps[64:128, :])
                # duplicate Q^T into partitions 64:128 for row-tiled scores
                nc.vector.tensor_copy(qt[64:128, c0:c1], ps[0:64, :])

            def vt_block(st0, st1):  # V^T -> V via PE transpose
                for st in range(st0, st1):
                    pvt = ppx.tile([128, D], dt.bfloat16, tag="px")
                    nc.tensor.transpose(
                        pvt[:],
                        vtsb[64:128, 128 * st : 128 * (st + 1)],
                        idb_sb[64:128, 64:128],
                    )
                    nc.vector.tensor_copy(vp[:, st, 0:D], pvt[:])

            # --- t-loop slot machinery: AV deferred one slot behind ---
            pending = []

            def flush_av():
                if not pending:
                    return
                pt, t, acc = pending.pop()
                for nb in range(2):
                    nc.tensor.matmul(
                        acc[:, 512 * nb : 512 * (nb + 1)],
                        vp[:, t, :],
                        pt[:, 512 * nb : 512 * (nb + 1)],
                        start=(t == 0),
                        stop=(t == JT - 1),
                    )

            def t_slot(t, acc, ih):
                ps = psco.tile([128, 1024], dt.float32, tag="ps")
                # row-tiled pair: groups 0/1 run concurrently (contraction 64)
                nc.tensor.matmul(
                    ps[:, 0:512],
                    qt[0:64, 128 * t : 128 * (t + 1)],
                    kt[0:64, 1024 * ih : 1024 * ih + 512],
                    start=True,
                    stop=True,
                )
                nc.tensor.matmul(
                    ps[:, 512:1024],
                    qt[64:128, 128 * t : 128 * (t + 1)],
                    kt[64:128, 1024 * ih + 512 : 1024 * ih + 1024],
                    start=True,
                    stop=True,
                )
                flush_av()
                pt = ptile.tile([128, 1024], dt.bfloat16)
                if t < nfull:
                    nc.scalar.activation(pt[:], ps[:], AF.Exp, scale=0.125)
                else:
                    nc.scalar.activation(
                        pt[:], ps[:], AF.Exp, bias=mb_sb[:, t : t + 1], scale=0.125
                    )
                pending.append((pt, t, acc))

            def acc_to_sb(acc, ih):  # copy PSUM acc to SBUF, freeing pacc
                nc.vector.tensor_copy(acc_sb[:, ih, 0:512], acc[:, 0:512])
                nc.vector.tensor_copy(acc_sb[:, ih, 512:1024], acc[:, 512:1024])
                # denominator row to partition 0 (row group of the ones
                # stationary in the broadcast matmul)
                nc.vector.tensor_copy(den_sb[:, ih, :], acc_sb[64:65, ih, :])

            def finale_steps(ih):
                # divide in transposed space; store out^T with 4KB lines
                steps = []
                for half in range(2):
                    c0, c1 = 512 * half, 512 * (half + 1)

                    def bcast(h0=c0, h1=c1, i=ih):
                        pd = ppx.tile([128, 512], dt.float32, tag="px")
                        nc.tensor.matmul(
                            pd[0:64, :],
                            onesb[0:1, 0:64],
                            den_sb[0:1, i, h0:h1],
                            start=True,
                            stop=True,
                        )
                        nc.vector.reciprocal(rc_sb[:, i, h0:h1], pd[0:64, :])

                    def mult(h0=c0, h1=c1, i=ih):
                        nc.vector.tensor_mul(
                            outT[:, i, h0:h1],
                            acc_sb[0:64, i, h0:h1],
                            rc_sb[:, i, h0:h1],
                        )

                    steps.append(bcast)
                    steps.append(mult)

                def store(i=ih):
                    nc.gpsimd.dma_start(
                        outt[:, 1024 * i : 1024 * (i + 1)], outT[:, i, :]
                    )

                steps.append(store)
                return steps

            # ---- pass A (i-half 0) interleaved with the projections ----
            big_loads()
            accA = pacc.tile([D + 1, 1024], dt.float32, tag="acc")
            tA = lambda t: t_slot(t, accA, 0)
            # PE warmup (128-wide junk) while the first slices stream in
            pw = ppx.tile([128, 128], dt.float32, tag="px")
            for _ in range(N_WARM):
                nc.tensor.matmul(
                    pw[:], wtile[:], wtile[:], start=True, stop=True
                )
            proj_k_pair(0)
            proj_qv(0)
            vt_cover = qblocks[0][1] // 128
            vt_block(0, vt_cover)
            next_t = 0
            units = []
            for i in range(1, len(qblocks)):
                units.append(("qv", i))
                if i == 2:
                    units.append(("k", 1))
            if ("k", 1) not in units:
                units.append(("k", 1))
            for kind, i in units:
                # emit already-runnable slots BEFORE the next proj unit:
                # the PE queue is in-order, so a proj waiting on its DMA
                # must not head-of-line-block ready slots
                tgt = min(vt_cover, next_t + 2)
                while next_t < tgt:
                    tA(next_t)
                    next_t += 1
                if kind == "qv":
                    proj_qv(i)
                    new_cover = qblocks[i][1] // 128
                    vt_block(vt_cover, new_cover)
                    vt_cover = new_cover
                else:
                    proj_k_pair(i)
            while next_t < JT:
                tA(next_t)
                next_t += 1
            # A's last AV stays pending: it flushes inside pass B's slot 0,
            # so pass-B scores aren't serialized behind it

            # ---- pass B (i-half 1), finale A interleaved into its slack ----
            accB = pacc.tile([D + 1, 1024], dt.float32, tag="acc")
            finA = None
            for t in range(JT):
                t_slot(t, accB, 1)
                if t == 0:
                    acc_to_sb(accA, 0)  # frees pacc banks for accB
                    finA = finale_steps(0)
                elif finA:
                    finA.pop(0)()
            while finA:
                finA.pop(0)()
            flush_av()
            acc_to_sb(accB, 1)
            for step in finale_steps(1):
                step()

    nc.compile()
    return nc


def _in_maps(x, mask, Wk, Wq, Wv):
    import ml_dtypes

    bf16 = ml_dtypes.bfloat16
    w4 = np.ascontiguousarray(
        np.concatenate([Wq.T, Wv.T, Wk.T, Wk.T], axis=1).astype(bf16)
    )
    idb = np.eye(128, dtype=bf16)
    nk = [int((mask[b] != 0).sum()) for b in range(B)]
    J = max(J_MIN, -(-max(nk) // 128) * 128)
    nfull = min(nk) // 128
    JT = J // 128
    xtq_b, mb_b = [], []
    for b in range(B):
        idx = np.flatnonzero(mask[b] != 0)
        xt = np.zeros((H, J), dtype=bf16)
        xt[:, : len(idx)] = x[b].T[:, idx].astype(bf16)
        xtq_b.append(xt)
        mbv = np.full(J, np.float32(NEG), dtype=np.float32)
        mbv[: len(idx)] = 0.0
        mb_b.append(np.ascontiguousarray(mbv.reshape(JT, 128).T))
    maps = []
    for c in range(N_CORES):
        b, half = c // 2, c % 2
        xtk = np.ascontiguousarray(x[b, half * SC : (half + 1) * SC].T.astype(bf16))
        maps.append(
            {
                "xtk": xtk,
                "xtq": xtq_b[b],
                "w4": w4,
                "mb": mb_b[b],
                "idb": idb,
            }
        )
    return maps, (J, nfull)


def kernel(x, mask, Wk, Wq, Wv):
    from concourse.bass_utils import run_bass_kernel_spmd

    maps, key = _in_maps(x, mask, Wk, Wq, Wv)
    if key not in _CACHE:
        _CACHE[key] = _build(*key)
    nc = _CACHE[key]
    br = run_bass_kernel_spmd(nc, maps, list(range(N_CORES)))
    out = np.empty((B, S, D), dtype=np.float32)
    for c in range(N_CORES):
        b, half = c // 2, c % 2
        out[b, half * SC : (half + 1) * SC, :] = br.results[c]["outt"].T
    return out


# revision 14
# speedup vs baseline: 1.0825x; 1.0479x over previous
"""Trainium2 Bass kernel for nn_AttentionHead (B=4, S=4096, H=1024, D=64).

Reference computation (note the unusual K-first ordering):
    K = x @ Wk.T; Q = x @ Wq.T; V = x @ Wv.T            [B,S,D]
    scores[b,i,j] = (K[b,i] . Q[b,j]) / sqrt(D)         [B,S,S]
    scores[:, :, j] = -1e12 where mask[:, j] == 0
    out = softmax(scores, axis=2) @ V                   [B,S,S] @ [B,S,D]

Key structural choices (v2 — DMA/fill-phase and finale overhaul):
  - Masked j-columns get softmax weight EXACTLY 0 (exp underflows), so the
    host drops them up front: the query/value axis is compacted from the
    mask (~2048 of 4096 survive) and padded to a fixed J. This halves the
    scores/exp/AV work, which dominates.
  - x^T ships pre-transposed in bf16; all weights ship as one merged
    [Wq|Wv|Wk|Wk] tensor (one stationary gives Q^T+V^T, the other K^T
    duplicated for PE row tiling).
  - Scores use PE row tiling (contraction D=64): the two 512-wide score
    matmuls of a slot run concurrently on row groups 0/1.
  - The t-loop is ACT(exp)-paced (~1.3us per [128,1024] tile); everything
    else is scheduled to never stall the scalar engine: its queue carries
    only the dummy exp + two tiny loads.
  - DMA: five trigger queues (sync/scalar/gpsimd/vector/tensor). The
    critical fill set (w4, xk half A, xq block 0) is spread across queues
    so it lands in parallel with >=2KB lines; pass-B-only data (xk cols
    1024:2048) and late xq blocks queue behind. The warmup is 128-wide
    junk matmuls bridging trigger-issue to first-data.
  - AV matmuls are emitted one slot behind scores (PE queue is in-order;
    they must not head-of-line block waiting on exp).
  - Finale v2: out^T stays in transposed space. The softmax denominator
    row is broadcast to 64 partitions with a K=1 matmul against a ones
    stationary, reciprocal+multiply on DVE, and the [64, 1024] fp32 result
    is stored as out^T with 4KB lines (the host transposes back). This
    replaces 16 PE transposes + 2048-descriptor stores.

Sharding: 8 cores = 4 batches x 2 key-row halves of 2048. Core output is
out^T [64, 2048]; host reassembles.
"""

import numpy as np

B, S, H, D = 4, 4096, 1024, 64
N_CORES = 8
SC = S // 2  # key rows (output rows) per core
HC = H // 128  # contraction chunks
J_MIN = 1024  # floor for the padded, mask-compacted query-column count
NEG = -30000.0
N_WARM = 40

_CACHE = {}


def _build(J, nfull):
    # nfull: query tiles [0, nfull) are fully kept for EVERY batch, so their
    # exp needs no mask bias (saves the ACT per-partition bias read).
    import concourse.tile as tile
    from concourse import bacc, mybir

    dt = mybir.dt
    AF = mybir.ActivationFunctionType
    JT = J // 128
    # first block narrow (256) so slot 0 starts early; 512-wide after
    qblocks = [(0, min(256, J))]
    c = 256
    while c < J:
        qblocks.append((c, min(c + 512, J)))
        c += 512

    nc = bacc.Bacc(
        "TRN2", target_bir_lowering=False, debug=False, num_devices=N_CORES
    )
    xtk = nc.dram_tensor("xtk", [H, SC], dt.bfloat16, kind="ExternalInput").ap()
    xtq = nc.dram_tensor("xtq", [H, J], dt.bfloat16, kind="ExternalInput").ap()
    w4 = nc.dram_tensor("w4", [H, 4 * D], dt.bfloat16, kind="ExternalInput").ap()
    mb = nc.dram_tensor("mb", [128, JT], dt.float32, kind="ExternalInput").ap()
    idb = nc.dram_tensor("idb", [128, 128], dt.bfloat16, kind="ExternalInput").ap()
    outt = nc.dram_tensor("outt", [D, SC], dt.float32, kind="ExternalOutput").ap()

    xtk_r = xtk.rearrange("(c p) s -> p c s", p=128)
    xtq_r = xtq.rearrange("(c p) s -> p c s", p=128)
    w4_r = w4.rearrange("(c p) d -> p c d", p=128)

    with (
        tile.TileContext(nc) as tc,
        tc.tile_pool(name="persist", bufs=1) as persist,
        tc.tile_pool(name="ptile", bufs=8) as ptile,
    ):
        qt = persist.tile([128, J], dt.bfloat16)  # Q^T duplicated rows 0:64/64:128
        kt = persist.tile([128, SC], dt.bfloat16)  # K^T duplicated rows 0:64/64:128
        vtsb = persist.tile([128, J], dt.bfloat16)  # rows 64:128 = V^T
        vp = persist.tile([128, JT, D + 1], dt.bfloat16)
        mb_sb = persist.tile([128, JT], dt.float32)
        idb_sb = persist.tile([128, 128], dt.bfloat16)
        wsb = persist.tile([128, HC, 4 * D], dt.bfloat16)
        xk_sb = persist.tile([128, HC, SC], dt.bfloat16)
        xq_sb = persist.tile([128, HC, J], dt.bfloat16)
        onesb = persist.tile([D + 1, D], dt.float32)
        wtile = persist.tile([128, 128], dt.bfloat16)
        acc_sb = persist.tile([D + 1, 2, 1024], dt.float32)
        rc_sb = persist.tile([D, 2, 1024], dt.float32)
        outT = persist.tile([D, 2, 1024], dt.float32)

        nc.vector.memset(wtile[:], 0.0)
        nc.vector.memset(vp[:, :, D], 1.0)
        # ones stationary lives at partition 64: same row group as the
        # denominator row of acc_sb it broadcasts in the finale matmul
        nc.vector.memset(onesb[64:65, :], 1.0)

        with (
            tc.tile_pool(name="psco", bufs=2, space="PSUM") as psco,
            tc.tile_pool(name="ppx", bufs=2, space="PSUM") as ppx,
            tc.tile_pool(name="pacc", bufs=1, space="PSUM") as pacc,
        ):
            dummy = persist.tile([128, 1], dt.float32)
            nc.scalar.activation(dummy[:], wtile[:, 0:1], AF.Exp)

            # --- DMA queue plans ---
            # Rate-weighted (gpsimd SWDGE fastest, scalar HWDGE slowest);
            # the scalar engine's triggers all precede the exp stream.
            # xq ranges: narrow first range matching qblocks[0], 1024 after
            xqranges = [(0, qblocks[0][1])]
            c = qblocks[0][1]
            while c < J:
                xqranges.append((c, min(c + 1024, J)))
                c += 1024

            def big_loads():
                nc.scalar.dma_start(mb_sb[:], mb[:])
                nc.scalar.dma_start(idb_sb[:], idb[:])
                nc.sync.dma_start(wsb[:, 0:4, :], w4_r[:, 0:4, :])
                nc.gpsimd.dma_start(wsb[:, 4:8, :], w4_r[:, 4:8, :])
                # xk half A: landing order 7,0,1,2,3,4,5,6 (KORDER below)
                nc.scalar.dma_start(xk_sb[:, 7:8, 0:1024], xtk_r[:, 7:8, 0:1024])
                nc.gpsimd.dma_start(xk_sb[:, 0:2, 0:1024], xtk_r[:, 0:2, 0:1024])
                nc.gpsimd.dma_start(xk_sb[:, 2:4, 0:1024], xtk_r[:, 2:4, 0:1024])
                nc.sync.dma_start(xk_sb[:, 4:7, 0:1024], xtk_r[:, 4:7, 0:1024])
                r0, r1 = xqranges[0]
                nc.scalar.dma_start(xq_sb[:, :, r0:r1], xtq_r[:, :, r0:r1])
                for c0, c1 in xqranges[1:]:
                    nc.scalar.dma_start(
                        xq_sb[:, 0:2, c0:c1], xtq_r[:, 0:2, c0:c1]
                    )
                    nc.sync.dma_start(xq_sb[:, 2:5, c0:c1], xtq_r[:, 2:5, c0:c1])
                    nc.gpsimd.dma_start(
                        xq_sb[:, 5:8, c0:c1], xtq_r[:, 5:8, c0:c1]
                    )
                nc.sync.dma_start(xk_sb[:, 0:4, 1024:2048], xtk_r[:, 0:4, 1024:2048])
                nc.gpsimd.dma_start(xk_sb[:, 4:8, 1024:2048], xtk_r[:, 4:8, 1024:2048])

            # --- PE work generators ---
            KORDER = [7, 0, 1, 2, 3, 4, 5, 6]  # xk half-A landing order

            def junk(n):  # HAM-warmth filler on the PE
                jp = psco.tile([128, 1024], dt.float32, tag="ps")
                for _ in range(n):
                    nc.tensor.matmul(
                        jp[:, 0:128], wtile[:], wtile[:], start=True, stop=True
                    )

            def proj_k_pair(sb, order, sprinkle=False):
                # kt blocks [1024*sb, 1024*sb+1024), per-hc interleaved
                c0 = 1024 * sb
                psL = ppx.tile([128, 512], dt.float32, tag="px")
                psR = ppx.tile([128, 512], dt.float32, tag="px")
                for n, hc in enumerate(order):
                    nc.tensor.matmul(
                        psL[:],
                        wsb[:, hc, 128:256],
                        xk_sb[:, hc, c0 : c0 + 512],
                        start=(n == 0),
                        stop=(n == HC - 1),
                    )
                    nc.tensor.matmul(
                        psR[:],
                        wsb[:, hc, 128:256],
                        xk_sb[:, hc, c0 + 512 : c0 + 1024],
                        start=(n == 0),
                        stop=(n == HC - 1),
                    )
                    if sprinkle and n in (0, 2):
                        junk(8)  # bridge DMA-chase gaps, keep HAM warm
                nc.vector.tensor_copy(kt[:, c0 : c0 + 512], psL[:])
                nc.vector.tensor_copy(kt[:, c0 + 512 : c0 + 1024], psR[:])

            def proj_qv(bi):  # [Q^T; V^T] for one query-column block
                c0, c1 = qblocks[bi]
                ps = ppx.tile([128, c1 - c0], dt.float32, tag="px")
                for hc in range(HC):
                    nc.tensor.matmul(
                        ps[:],
                        wsb[:, hc, 0:128],
                        xq_sb[:, hc, c0:c1],
                        start=(hc == 0),
                        stop=(hc == HC - 1),
                    )
                nc.vector.tensor_copy(qt[0:64, c0:c1], ps[0:64, :])
                nc.vector.tensor_copy(vtsb[64:128, c0:c1], ps[64:128, :])
                # duplicate Q^T into partitions 64:128 for row-tiled scores
                nc.vector.tensor_copy(qt[64:128, c0:c1], ps[0:64, :])

            def vt_block(st0, st1):  # V^T -> V via PE transpose
                for st in range(st0, st1):
                    pvt = ppx.tile([128, D], dt.bfloat16, tag="px")
                    nc.tensor.transpose(
                        pvt[:],
                        vtsb[64:128, 128 * st : 128 * (st + 1)],
                        idb_sb[64:128, 64:128],
                    )
                    nc.vector.tensor_copy(vp[:, st, 0:D], pvt[:])

            # --- t-loop slot machinery: AV deferred one slot behind ---
            pending = []

            def flush_av():
                if not pending:
                    return
                pt, t, acc = pending.pop()
                for nb in range(2):
                    nc.tensor.matmul(
                        acc[:, 512 * nb : 512 * (nb + 1)],
                        vp[:, t, :],
                        pt[:, 512 * nb : 512 * (nb + 1)],
                        start=(t == 0),
                        stop=(t == JT - 1),
                    )

            def t_slot(t, acc, ih):
                ps = psco.tile([128, 1024], dt.float32, tag="ps")
                # row-tiled pair: groups 0/1 run concurrently (contraction 64)
                nc.tensor.matmul(
                    ps[:, 0:512],
                    qt[0:64, 128 * t : 128 * (t + 1)],
                    kt[0:64, 1024 * ih : 1024 * ih + 512],
                    start=True,
                    stop=True,
                )
                nc.tensor.matmul(
                    ps[:, 512:1024],
                    qt[64:128, 128 * t : 128 * (t + 1)],
                    kt[64:128, 1024 * ih + 512 : 1024 * ih + 1024],
                    start=True,
                    stop=True,
                )
                flush_av()
                pt = ptile.tile([128, 1024], dt.bfloat16)
                if t < nfull:
                    nc.scalar.activation(pt[:], ps[:], AF.Exp, scale=0.125)
                else:
                    nc.scalar.activation(
                        pt[:], ps[:], AF.Exp, bias=mb_sb[:, t : t + 1], scale=0.125
                    )
                pending.append((pt, t, acc))

            def acc_to_sb(acc, ih):  # copy PSUM acc to SBUF, freeing pacc
                nc.vector.tensor_copy(acc_sb[:, ih, 0:512], acc[:, 0:512])
                nc.vector.tensor_copy(acc_sb[:, ih, 512:1024], acc[:, 512:1024])

            def finale_steps(ih):
                # divide in transposed space; store out^T with 4KB lines
                steps = []
                for half in range(2):
                    c0, c1 = 512 * half, 512 * (half + 1)

                    def bcast(h0=c0, h1=c1, i=ih):
                        # denominator row (partition 64) broadcast to 64
                        # partitions via K=1 matmul in row group 2
                        pd = ppx.tile([128, 512], dt.float32, tag="px")
                        nc.tensor.matmul(
                            pd[0:64, :],
                            onesb[64:65, :],
                            acc_sb[64:65, i, h0:h1],
                            start=True,
                            stop=True,
                        )
                        nc.vector.reciprocal_approx_fast(
                            rc_sb[:, i, h0:h1], pd[0:64, :]
                        )

                    def mult(h0=c0, h1=c1, i=ih):
                        nc.gpsimd.tensor_mul(
                            outT[:, i, h0:h1],
                            acc_sb[0:64, i, h0:h1],
                            rc_sb[:, i, h0:h1],
                        )

                    steps.append(bcast)
                    steps.append(mult)

                def store(i=ih):
                    nc.sync.dma_start(
                        outt[:, 1024 * i : 1024 * (i + 1)], outT[:, i, :]
                    )

                steps.append(store)
                return steps

            # ---- pass A (i-half 0) interleaved with the projections ----
            big_loads()
            accA = pacc.tile([D + 1, 1024], dt.float32, tag="acc")
            tA = lambda t: t_slot(t, accA, 0)
            # PE warmup (128-wide junk) while the first slices stream in
            junk(N_WARM)
            proj_k_pair(0, KORDER, sprinkle=True)
            proj_qv(0)
            vt_cover = qblocks[0][1] // 128
            vt_block(0, vt_cover)
            next_t = 0
            for i in range(1, len(qblocks)):
                # emit already-runnable slots BEFORE the next proj unit:
                # the PE queue is in-order, so a proj waiting on its DMA
                # must not head-of-line-block ready slots
                tgt = min(vt_cover, next_t + 2)
                while next_t < tgt:
                    tA(next_t)
                    next_t += 1
                proj_qv(i)
                new_cover = qblocks[i][1] // 128
                vt_block(vt_cover, new_cover)
                vt_cover = new_cover
            # kt half B late: its xk data is last in the DMA queues
            while next_t < min(10, JT):
                tA(next_t)
                next_t += 1
            proj_k_pair(1, list(range(HC)))
            while next_t < JT:
                tA(next_t)
                next_t += 1
            flush_av()
            acc_to_sb(accA, 0)

            # ---- pass B (i-half 1), finale A interleaved into its slack ----
            accB = pacc.tile([D + 1, 1024], dt.float32, tag="acc")
            finA = finale_steps(0)
            for t in range(JT):
                t_slot(t, accB, 1)
                if finA and t >= 3 and t % 2 == 1:
                    finA.pop(0)()
            while finA:
                finA.pop(0)()
            flush_av()
            acc_to_sb(accB, 1)
            for step in finale_steps(1):
                step()

    nc.compile()
    return nc


def _in_maps(x, mask, Wk, Wq, Wv):
    import ml_dtypes

    bf16 = ml_dtypes.bfloat16
    w4 = np.ascontiguousarray(
        np.concatenate([Wq.T, Wv.T, Wk.T, Wk.T], axis=1).astype(bf16)
    )
    idb = np.eye(128, dtype=bf16)
    nk = [int((mask[b] != 0).sum()) for b in range(B)]
    J = max(J_MIN, -(-max(nk) // 128) * 128)
    nfull = min(nk) // 128
    JT = J // 128
    xtq_b, mb_b = [], []
    for b in range(B):
        idx = np.flatnonzero(mask[b] != 0)
        xt = np.zeros((H, J), dtype=bf16)
        xt[:, : len(idx)] = x[b].T[:, idx].astype(bf16)
        xtq_b.append(xt)
        mbv = np.full(J, np.float32(NEG), dtype=np.float32)
        mbv[: len(idx)] = 0.0
        mb_b.append(np.ascontiguousarray(mbv.reshape(JT, 128).T))
    maps = []
    for c in range(N_CORES):
        b, half = c // 2, c % 2
        xtk = np.ascontiguousarray(x[b, half * SC : (half + 1) * SC].T.astype(bf16))
        maps.append(
            {
                "xtk": xtk,
                "xtq": xtq_b[b],
                "w4": w4,
                "mb": mb_b[b],
                "idb": idb,
            }
        )
    return maps, (J, nfull)


def kernel(x, mask, Wk, Wq, Wv):
    from concourse.bass_utils import run_bass_kernel_spmd

    maps, key = _in_maps(x, mask, Wk, Wq, Wv)
    if key not in _CACHE:
        _CACHE[key] = _build(*key)
    nc = _CACHE[key]
    br = run_bass_kernel_spmd(nc, maps, list(range(N_CORES)))
    out = np.empty((B, S, D), dtype=np.float32)
    for c in range(N_CORES):
        b, half = c // 2, c % 2
        out[b, half * SC : (half + 1) * SC, :] = br.results[c]["outt"].T
    return out


# revision 21
# speedup vs baseline: 1.1651x; 1.0764x over previous
"""Trainium2 Bass kernel for nn_AttentionHead (B=4, S=4096, H=1024, D=64).

Reference computation (note the unusual K-first ordering):
    K = x @ Wk.T; Q = x @ Wq.T; V = x @ Wv.T            [B,S,D]
    scores[b,i,j] = (K[b,i] . Q[b,j]) / sqrt(D)         [B,S,S]
    scores[:, :, j] = -1e12 where mask[:, j] == 0
    out = softmax(scores, axis=2) @ V                   [B,S,S] @ [B,S,D]

Key structural choices (v2 — DMA/fill-phase and finale overhaul):
  - Masked j-columns get softmax weight EXACTLY 0 (exp underflows), so the
    host drops them up front: the query/value axis is compacted from the
    mask (~2048 of 4096 survive) and padded to a fixed J. This halves the
    scores/exp/AV work, which dominates.
  - x^T ships pre-transposed in bf16; all weights ship as one merged
    [Wq|Wv|Wk|Wk] tensor (one stationary gives Q^T+V^T, the other K^T
    duplicated for PE row tiling).
  - Scores use PE row tiling (contraction D=64): the two 512-wide score
    matmuls of a slot run concurrently on row groups 0/1.
  - The t-loop is ACT(exp)-paced (~1.3us per [128,1024] tile); everything
    else is scheduled to never stall the scalar engine: its queue carries
    only the dummy exp + two tiny loads.
  - DMA: five trigger queues (sync/scalar/gpsimd/vector/tensor). The
    critical fill set (w4, xk half A, xq block 0) is spread across queues
    so it lands in parallel with >=2KB lines; pass-B-only data (xk cols
    1024:2048) and late xq blocks queue behind. The warmup is 128-wide
    junk matmuls bridging trigger-issue to first-data.
  - AV matmuls are emitted one slot behind scores (PE queue is in-order;
    they must not head-of-line block waiting on exp).
  - Finale v2: out^T stays in transposed space. The softmax denominator
    row is broadcast to 64 partitions with a K=1 matmul against a ones
    stationary, reciprocal+multiply on DVE, and the [64, 1024] fp32 result
    is stored as out^T with 4KB lines (the host transposes back). This
    replaces 16 PE transposes + 2048-descriptor stores.

Sharding: 8 cores = 4 batches x 2 key-row halves of 2048. Core output is
out^T [64, 2048]; host reassembles.
"""

import numpy as np

B, S, H, D = 4, 4096, 1024, 64
N_CORES = 8
SC = S // 2  # key rows (output rows) per core
HC = H // 128  # contraction chunks
J_MIN = 1024  # floor for the padded, mask-compacted query-column count
NEG = -30000.0
N_WARM = 40

_CACHE = {}


def _build(J, nfull):
    # nfull: query tiles [0, nfull) are fully kept for EVERY batch, so their
    # exp needs no mask bias (saves the ACT per-partition bias read).
    import concourse.tile as tile
    from concourse import bacc, mybir

    dt = mybir.dt
    AF = mybir.ActivationFunctionType
    JT = J // 128
    qblocks = [(c0, min(c0 + 512, J)) for c0 in range(0, J, 512)]

    nc = bacc.Bacc(
        "TRN2", target_bir_lowering=False, debug=False, num_devices=N_CORES
    )
    xtk = nc.dram_tensor("xtk", [H, SC], dt.bfloat16, kind="ExternalInput").ap()
    xtq = nc.dram_tensor("xtq", [H, J], dt.bfloat16, kind="ExternalInput").ap()
    w4 = nc.dram_tensor("w4", [H, 4 * D], dt.bfloat16, kind="ExternalInput").ap()
    mb = nc.dram_tensor("mb", [128, JT], dt.float32, kind="ExternalInput").ap()
    outt = nc.dram_tensor("outt", [D, SC], dt.float32, kind="ExternalOutput").ap()

    xtk_r = xtk.rearrange("(c p) s -> p c s", p=128)
    xtq_r = xtq.rearrange("(c p) s -> p c s", p=128)
    w4_r = w4.rearrange("(c p) d -> p c d", p=128)

    with (
        tile.TileContext(nc) as tc,
        tc.tile_pool(name="persist", bufs=1) as persist,
        tc.tile_pool(name="ptile", bufs=8) as ptile,
    ):
        qt = persist.tile([128, J], dt.bfloat16)  # Q^T duplicated rows 0:64/64:128
        kt = persist.tile([128, SC], dt.bfloat16)  # K^T duplicated rows 0:64/64:128
        vtsb = persist.tile([128, J], dt.bfloat16)  # rows 64:128 = V^T
        vp = persist.tile([128, JT, D + 1], dt.bfloat16)
        mb_sb = persist.tile([128, JT], dt.float32)
        idb_sb = persist.tile([128, 128], dt.bfloat16)
        wsb = persist.tile([128, HC, 4 * D], dt.bfloat16)
        xk_sb = persist.tile([128, HC, SC], dt.bfloat16)
        xq_sb = persist.tile([128, HC, J], dt.bfloat16)
        onesb = persist.tile([D + 1, D], dt.float32)
        wtile = persist.tile([128, 128], dt.bfloat16)
        acc_sb = persist.tile([D + 1, 2, 1024], dt.float32)
        rc_sb = persist.tile([D, 2, 1024], dt.float32)
        outT = persist.tile([D, 2, 1024], dt.float32)

        nc.vector.memset(wtile[:], 0.0)
        nc.vector.memset(vp[:, :, D], 1.0)
        # ones stationary lives at partition 64: same row group as the
        # denominator row of acc_sb it broadcasts in the finale matmul
        nc.vector.memset(onesb[64:65, :], 1.0)
        # identity for the V^T->V PE transposes, built on-chip (a DMA'd
        # identity costs thousands of tiny descriptors)
        ia = persist.tile([128, 128], dt.int16)
        nc.gpsimd.iota(ia[:], [[1, 128]], base=0, channel_multiplier=-1)
        nc.gpsimd.tensor_scalar(
            idb_sb[:], ia[:], 0, None, mybir.AluOpType.is_equal
        )

        with (
            tc.tile_pool(name="psco", bufs=2, space="PSUM") as psco,
            tc.tile_pool(name="ppx", bufs=2, space="PSUM") as ppx,
            tc.tile_pool(name="pacc", bufs=1, space="PSUM") as pacc,
        ):
            dummy = persist.tile([128, 1], dt.float32)
            nc.scalar.activation(dummy[:], wtile[:, 0:1], AF.Exp)

            # --- DMA queue plans ---
            # Queue rate is descriptor-limited (~30-50ns/line): only >=1KB
            # lines for bulk, nothing tiny ahead of critical data. The
            # critical set (w4, xk half A, xq block 0) is split across all
            # three queues; mb (68B lines) rides late on scalar.
            xqranges = [(0, min(512, J))]
            c = 512
            while c < J:
                xqranges.append((c, min(c + 1024, J)))
                c += 1024

            def big_loads():
                nc.sync.dma_start(wsb[:, 0:4, :], w4_r[:, 0:4, :])
                nc.gpsimd.dma_start(wsb[:, 4:8, :], w4_r[:, 4:8, :])
                # xk half A: landing order 7,0,1,2,3,4,5,6 (KORDER below)
                nc.scalar.dma_start(xk_sb[:, 7:8, 0:1024], xtk_r[:, 7:8, 0:1024])
                nc.gpsimd.dma_start(xk_sb[:, 0:2, 0:1024], xtk_r[:, 0:2, 0:1024])
                nc.gpsimd.dma_start(xk_sb[:, 2:4, 0:1024], xtk_r[:, 2:4, 0:1024])
                nc.sync.dma_start(xk_sb[:, 4:6, 0:1024], xtk_r[:, 4:6, 0:1024])
                nc.sync.dma_start(xk_sb[:, 6:7, 0:1024], xtk_r[:, 6:7, 0:1024])
                r0, r1 = xqranges[0]
                nc.scalar.dma_start(xq_sb[:, 0:3, r0:r1], xtq_r[:, 0:3, r0:r1])
                nc.sync.dma_start(xq_sb[:, 3:6, r0:r1], xtq_r[:, 3:6, r0:r1])
                nc.gpsimd.dma_start(xq_sb[:, 6:8, r0:r1], xtq_r[:, 6:8, r0:r1])
                for c0, c1 in xqranges[1:]:
                    nc.scalar.dma_start(
                        xq_sb[:, 0:3, c0:c1], xtq_r[:, 0:3, c0:c1]
                    )
                    nc.sync.dma_start(xq_sb[:, 3:6, c0:c1], xtq_r[:, 3:6, c0:c1])
                    nc.gpsimd.dma_start(
                        xq_sb[:, 6:8, c0:c1], xtq_r[:, 6:8, c0:c1]
                    )
                nc.scalar.dma_start(mb_sb[:], mb[:])
                nc.sync.dma_start(xk_sb[:, 0:4, 1024:2048], xtk_r[:, 0:4, 1024:2048])
                nc.gpsimd.dma_start(xk_sb[:, 4:8, 1024:2048], xtk_r[:, 4:8, 1024:2048])

            # --- PE work generators ---
            KORDER = [7, 0, 1, 2, 3, 4, 5, 6]  # xk half-A landing order

            def junk(n):  # HAM-warmth filler on the PE
                jp = psco.tile([128, 1024], dt.float32, tag="ps")
                for _ in range(n):
                    nc.tensor.matmul(
                        jp[:, 0:128], wtile[:], wtile[:], start=True, stop=True
                    )

            def proj_k_pair(sb, order, sprinkle=False):
                # kt blocks [1024*sb, 1024*sb+1024), per-hc interleaved
                c0 = 1024 * sb
                psL = ppx.tile([128, 512], dt.float32, tag="px")
                psR = ppx.tile([128, 512], dt.float32, tag="px")
                for n, hc in enumerate(order):
                    nc.tensor.matmul(
                        psL[:],
                        wsb[:, hc, 128:256],
                        xk_sb[:, hc, c0 : c0 + 512],
                        start=(n == 0),
                        stop=(n == HC - 1),
                    )
                    nc.tensor.matmul(
                        psR[:],
                        wsb[:, hc, 128:256],
                        xk_sb[:, hc, c0 + 512 : c0 + 1024],
                        start=(n == 0),
                        stop=(n == HC - 1),
                    )
                    if sprinkle and n in (0, 2):
                        junk(8)  # bridge DMA-chase gaps, keep HAM warm
                nc.vector.tensor_copy(kt[:, c0 : c0 + 512], psL[:])
                nc.vector.tensor_copy(kt[:, c0 + 512 : c0 + 1024], psR[:])

            def proj_qv(bi):  # [Q^T; V^T] for one query-column block
                c0, c1 = qblocks[bi]
                ps = ppx.tile([128, c1 - c0], dt.float32, tag="px")
                for hc in range(HC):
                    nc.tensor.matmul(
                        ps[:],
                        wsb[:, hc, 0:128],
                        xq_sb[:, hc, c0:c1],
                        start=(hc == 0),
                        stop=(hc == HC - 1),
                    )
                nc.vector.tensor_copy(qt[0:64, c0:c1], ps[0:64, :])
                nc.vector.tensor_copy(vtsb[64:128, c0:c1], ps[64:128, :])
                # duplicate Q^T into partitions 64:128 for row-tiled scores
                nc.vector.tensor_copy(qt[64:128, c0:c1], ps[0:64, :])

            def vt_block(st0, st1):  # V^T -> V via PE transpose
                for st in range(st0, st1):
                    pvt = ppx.tile([128, D], dt.bfloat16, tag="px")
                    nc.tensor.transpose(
                        pvt[:],
                        vtsb[64:128, 128 * st : 128 * (st + 1)],
                        idb_sb[64:128, 64:128],
                    )
                    nc.vector.tensor_copy(vp[:, st, 0:D], pvt[:])

            # --- t-loop slot machinery: AV deferred one slot behind ---
            pending = []

            def flush_av():
                if not pending:
                    return
                pt, t, acc = pending.pop()
                for nb in range(2):
                    nc.tensor.matmul(
                        acc[:, 512 * nb : 512 * (nb + 1)],
                        vp[:, t, :],
                        pt[:, 512 * nb : 512 * (nb + 1)],
                        start=(t == 0),
                        stop=(t == JT - 1),
                    )

            def t_slot(t, acc, ih):
                ps = psco.tile([128, 1024], dt.float32, tag="ps")
                # row-tiled pair: groups 0/1 run concurrently (contraction 64)
                nc.tensor.matmul(
                    ps[:, 0:512],
                    qt[0:64, 128 * t : 128 * (t + 1)],
                    kt[0:64, 1024 * ih : 1024 * ih + 512],
                    start=True,
                    stop=True,
                )
                nc.tensor.matmul(
                    ps[:, 512:1024],
                    qt[64:128, 128 * t : 128 * (t + 1)],
                    kt[64:128, 1024 * ih + 512 : 1024 * ih + 1024],
                    start=True,
                    stop=True,
                )
                flush_av()
                pt = ptile.tile([128, 1024], dt.bfloat16)
                if t < nfull:
                    nc.scalar.activation(pt[:], ps[:], AF.Exp, scale=0.125)
                else:
                    nc.scalar.activation(
                        pt[:], ps[:], AF.Exp, bias=mb_sb[:, t : t + 1], scale=0.125
                    )
                pending.append((pt, t, acc))

            def acc_to_sb(acc, ih):  # copy PSUM acc to SBUF, freeing pacc
                nc.vector.tensor_copy(acc_sb[:, ih, 0:512], acc[:, 0:512])
                nc.vector.tensor_copy(acc_sb[:, ih, 512:1024], acc[:, 512:1024])

            def finale_steps(ih):
                # divide in transposed space; store out^T with 4KB lines
                steps = []
                for half in range(2):
                    c0, c1 = 512 * half, 512 * (half + 1)

                    def bcast(h0=c0, h1=c1, i=ih):
                        # denominator row (partition 64) broadcast to 64
                        # partitions via K=1 matmul in row group 2
                        pd = ppx.tile([128, 512], dt.float32, tag="px")
                        nc.tensor.matmul(
                            pd[0:64, :],
                            onesb[64:65, :],
                            acc_sb[64:65, i, h0:h1],
                            start=True,
                            stop=True,
                        )
                        nc.vector.reciprocal_approx_fast(
                            rc_sb[:, i, h0:h1], pd[0:64, :]
                        )

                    def mult(h0=c0, h1=c1, i=ih):
                        nc.gpsimd.tensor_mul(
                            outT[:, i, h0:h1],
                            acc_sb[0:64, i, h0:h1],
                            rc_sb[:, i, h0:h1],
                        )

                    def store(h0=c0, h1=c1, i=ih):
                        nc.sync.dma_start(
                            outt[:, 1024 * i + h0 : 1024 * i + h1],
                            outT[:, i, h0:h1],
                        )

                    steps.append(bcast)
                    steps.append(mult)
                    steps.append(store)
                return steps

            # ---- pass A (i-half 0) interleaved with the projections ----
            big_loads()
            accA = pacc.tile([D + 1, 1024], dt.float32, tag="acc")
            tA = lambda t: t_slot(t, accA, 0)
            # PE warmup (128-wide junk) while the first slices stream in
            junk(N_WARM)
            proj_k_pair(0, KORDER, sprinkle=True)
            proj_qv(0)
            vt_cover = qblocks[0][1] // 128
            vt_block(0, vt_cover)
            next_t = 0
            for i in range(1, len(qblocks)):
                # emit already-runnable slots BEFORE the next proj unit:
                # the PE queue is in-order, so a proj waiting on its DMA
                # must not head-of-line-block ready slots
                tgt = min(vt_cover, next_t + 2)
                while next_t < tgt:
                    tA(next_t)
                    next_t += 1
                proj_qv(i)
                new_cover = qblocks[i][1] // 128
                vt_block(vt_cover, new_cover)
                vt_cover = new_cover
            # kt half B late: its xk data is last in the DMA queues
            while next_t < min(10, JT):
                tA(next_t)
                next_t += 1
            proj_k_pair(1, list(range(HC)))
            while next_t < JT:
                tA(next_t)
                next_t += 1
            flush_av()
            acc_to_sb(accA, 0)

            # ---- pass B (i-half 1), finale A interleaved into its slack ----
            accB = pacc.tile([D + 1, 1024], dt.float32, tag="acc")
            finA = finale_steps(0)
            for t in range(JT):
                t_slot(t, accB, 1)
                if finA and t >= 3 and t % 2 == 1:
                    finA.pop(0)()
            while finA:
                finA.pop(0)()
            flush_av()
            acc_to_sb(accB, 1)
            for step in finale_steps(1):
                step()

    nc.compile()
    return nc


def _in_maps(x, mask, Wk, Wq, Wv):
    import ml_dtypes

    bf16 = ml_dtypes.bfloat16
    w4 = np.ascontiguousarray(
        np.concatenate([Wq.T, Wv.T, Wk.T, Wk.T], axis=1).astype(bf16)
    )
    nk = [int((mask[b] != 0).sum()) for b in range(B)]
    J = max(J_MIN, -(-max(nk) // 128) * 128)
    nfull = min(nk) // 128
    JT = J // 128
    xtq_b, mb_b = [], []
    for b in range(B):
        idx = np.flatnonzero(mask[b] != 0)
        xt = np.zeros((H, J), dtype=bf16)
        xt[:, : len(idx)] = x[b].T[:, idx].astype(bf16)
        xtq_b.append(xt)
        mbv = np.full(J, np.float32(NEG), dtype=np.float32)
        mbv[: len(idx)] = 0.0
        mb_b.append(np.ascontiguousarray(mbv.reshape(JT, 128).T))
    maps = []
    for c in range(N_CORES):
        b, half = c // 2, c % 2
        xtk = np.ascontiguousarray(x[b, half * SC : (half + 1) * SC].T.astype(bf16))
        maps.append(
            {
                "xtk": xtk,
                "xtq": xtq_b[b],
                "w4": w4,
                "mb": mb_b[b],
            }
        )
    return maps, (J, nfull)


def kernel(x, mask, Wk, Wq, Wv):
    from concourse.bass_utils import run_bass_kernel_spmd

    maps, key = _in_maps(x, mask, Wk, Wq, Wv)
    if key not in _CACHE:
        _CACHE[key] = _build(*key)
    nc = _CACHE[key]
    br = run_bass_kernel_spmd(nc, maps, list(range(N_CORES)))
    out = np.empty((B, S, D), dtype=np.float32)
    for c in range(N_CORES):
        b, half = c // 2, c % 2
        out[b, half * SC : (half + 1) * SC, :] = br.results[c]["outt"].T
    return out
